# revision 13
# baseline (speedup 1.0000x reference)
"""Trainium2 Bass kernel for the NRI CNNEncoder (gnn_message_passing).

Strategy
--------
8-way shard over the edge dimension E=1560: each core owns 5 receiver nodes
x 40 sender slots (the self-edge is computed as padding and discarded on the
host) = 200 local edges x B=4 batches = 800 edge-sequences per core.

Algebraic restructuring (all exact, eval-mode):
- conv1 is linear, so per-edge conv1(concat(send, recv)) = F_s[send] + F_r[recv]
  where F_s/F_r are convolutions of the 160 node sequences with the two halves
  of conv1_w: a 39x compute reduction on conv1.
- BatchNorm (eval) = per-channel positive-scale affine; it commutes with
  maxpool and folds into the following conv/matmul weights (bn1 -> conv2,
  bn2 -> convp/conva).
- convp (1x1) commutes with the attention-weighted temporal pooling, so it is
  applied after pooling: 44x less convp compute.
- ELU is computed as relu(x+b) + exp(min(x+b,0)) - 1; every "-1" is folded
  into the next layer's bias on the host.
- edge2node needs a cross-core reduction: one 80KB fp32 AllReduce.

Matmuls run in bf16 (fp32 PSUM accumulation); softmax in fp32.
"""

import os
import sys
import numpy as np

sys.path.insert(0, "/opt/trn_rl_repo")

import ml_dtypes

BF16 = ml_dtypes.bfloat16

# Problem constants (hardcoded; must match the reference).
B, N, T, D, H, O = 4, 40, 100, 4, 128, 2
E = N * (N - 1)          # 1560
BN_EPS = 1e-5
N_CORES = 8
RPC = N // N_CORES       # receivers per core = 5
EL = RPC * N             # local edges per core (incl. self padding) = 200
PT = T - 4               # conv1 output length = 96
PL = PT // 2             # pooled length = 48
CT = PL - 4              # conv2 output length = 44
CK = 20                  # conv1 contraction = D * K = 4*5
FSTRIP = 480             # F matmul strip (5 nodes x 96)
C2EDGES = 10             # edges per conv2/logits strip
C2STRIP = C2EDGES * CT   # 440


def _np_forward(inputs, rel_rec, rel_send, p):
    """Pure-numpy fp32 replica of the reference (fallback for inputs whose
    rel matrices do not have the NRI one-hot structure)."""
    x32 = inputs.astype(np.float32)
    rr = rel_rec.astype(np.float32)
    rs = rel_send.astype(np.float32)
    xf = x32.reshape(B, N, T * D)
    recv = np.einsum("en,bnf->bef", rr, xf).reshape(B * rr.shape[0], T, D)
    send = np.einsum("en,bnf->bef", rs, xf).reshape(B * rs.shape[0], T, D)
    x = np.concatenate([send.transpose(0, 2, 1), recv.transpose(0, 2, 1)], axis=1)

    def conv1d(x, w, b):
        k = w.shape[2]
        t_out = x.shape[2] - k + 1
        y = np.zeros((x.shape[0], w.shape[0], t_out), np.float32)
        for kk in range(k):
            y += np.einsum("oc,nct->not", w[:, :, kk], x[:, :, kk:kk + t_out])
        return y + b[None, :, None]

    def bn(x, g, b, m, v):
        return (x - m[None, :, None]) / np.sqrt(v[None, :, None] + BN_EPS) \
            * g[None, :, None] + b[None, :, None]

    def elu(x):
        return np.where(x > 0, x, np.expm1(x))

    def mlp(x, w1, b1, w2, b2):
        h = elu(x @ w1 + b1)
        return elu(h @ w2 + b2)

    x = bn(np.maximum(conv1d(x, p["conv1_w"], p["conv1_b"]), 0.0),
           p["bn1_g"], p["bn1_b"], p["bn1_m"], p["bn1_v"])
    n_, c_, t_ = x.shape
    x = x.reshape(n_, c_, t_ // 2, 2).max(axis=-1)
    x = bn(np.maximum(conv1d(x, p["conv2_w"], p["conv2_b"]), 0.0),
           p["bn2_g"], p["bn2_b"], p["bn2_m"], p["bn2_v"])
    pred = conv1d(x, p["convp_w"], p["convp_b"])
    a = conv1d(x, p["conva_w"], p["conva_b"])
    a = np.exp(a - a.max(axis=2, keepdims=True))
    a = a / a.sum(axis=2, keepdims=True)
    x = (pred * a).mean(axis=2).reshape(B, -1, H)
    x = mlp(x, p["mlp1_w1"], p["mlp1_b1"], p["mlp1_w2"], p["mlp1_b2"])
    x_skip = x
    inc = np.einsum("en,beh->bnh", rr, x) / N
    x = mlp(inc, p["mlp2_w1"], p["mlp2_b1"], p["mlp2_w2"], p["mlp2_b2"])
    sn = np.einsum("en,bnh->beh", rs, x)
    rc = np.einsum("en,bnh->beh", rr, x)
    x = np.concatenate([sn, rc, x_skip], axis=2)
    x = mlp(x, p["mlp3_w1"], p["mlp3_b1"], p["mlp3_w2"], p["mlp3_b2"])
    return x @ p["fco_w"] + p["fco_b"]


def _nri_structure(rel_rec, rel_send):
    """If (rel_rec, rel_send) are the NRI fully-connected one-hot matrices,
    return edge_of[r][s] -> global edge index; else None."""
    if rel_rec.shape != (E, N) or rel_send.shape != (E, N):
        return None
    rec_i = np.argmax(rel_rec, axis=1)
    snd_i = np.argmax(rel_send, axis=1)
    eye = np.eye(N, dtype=rel_rec.dtype)
    if not (np.array_equal(rel_rec, eye[rec_i]) and
            np.array_equal(rel_send, eye[snd_i])):
        return None
    edge_of = {}
    for e in range(E):
        r, s = int(rec_i[e]), int(snd_i[e])
        if r == s or (r, s) in edge_of:
            return None
        edge_of[(r, s)] = e
    if len(edge_of) != E:
        return None
    return edge_of


_PROGRAM_CACHE = {}
TRACE = False          # test harness sets True to collect NTFF exec time
LAST_RESULT = None     # BassKernelResults of the last run (when TRACE)


def _build_program():
    """Build + compile the SPMD Bass program (cached per process)."""
    if "nc" in _PROGRAM_CACHE:
        return _PROGRAM_CACHE["nc"]

    import concourse.bacc as bacc
    import concourse.tile as tile
    from concourse import mybir
    from contextlib import ExitStack

    f32 = mybir.dt.float32
    bf16 = mybir.dt.bfloat16
    Alu = mybir.AluOpType
    Act = mybir.ActivationFunctionType

    nc = bacc.Bacc("TRN2", target_bir_lowering=False, debug=False,
                   num_devices=N_CORES)

    def din(name, shape, dt=bf16):
        return nc.dram_tensor(name, shape, dt, kind="ExternalInput").ap()

    p1 = din("p1", [B, CK, N * PT])
    p1r = din("p1r", [B, CK, RPC * PT])
    rel_r = din("rel_r", [EL, N], f32)
    rel_sT = din("rel_sT", [N, EL], f32)
    rel_rT = din("rel_rT", [N, EL], f32)
    w1s = din("w1s", [CK, H])
    w1r = din("w1r", [CK, H])
    w2 = [din(f"w2k{k}", [H, H]) for k in range(5)]
    wp = din("wp", [H, H], f32)
    wa = din("wa", [H, 1])
    w11 = din("w11", [H, H], f32)
    w12 = din("w12", [H, H], f32)
    w21 = din("w21", [H, H], f32)
    w22 = din("w22", [H, H], f32)
    w31a = din("w31a", [H, H], f32)
    w31b = din("w31b", [H, H], f32)
    w31c = din("w31c", [H, H], f32)
    w32 = din("w32", [H, H], f32)
    wfco = din("wfco", [H, O], f32)
    ident = din("ident", [H, H], f32)
    b1 = din("b1", [H, 1], f32)
    b2p = din("b2p", [H, 1], f32)
    b11 = din("b11", [H, 1], f32)
    b12 = din("b12", [H, 1], f32)
    b21 = din("b21", [H, 1], f32)
    b22 = din("b22", [H, 1], f32)
    b31 = din("b31", [H, 1], f32)
    b32 = din("b32", [H, 1], f32)
    bfco = din("bfco", [O, 1], f32)

    y = nc.dram_tensor("y", [B, EL, O], f32, kind="ExternalOutput").ap()
    cc_in = nc.dram_tensor("cc_in", [H, B * N], f32)
    cc_out = nc.dram_tensor("cc_out", [H, B * N], f32, addr_space="Shared")

    with tile.TileContext(nc) as tc:
        with ExitStack() as ctx:
            singles = ctx.enter_context(tc.tile_pool(name="singles", bufs=1))
            work = ctx.enter_context(tc.tile_pool(name="work", bufs=2))
            work3 = ctx.enter_context(tc.tile_pool(name="work3", bufs=3))
            psum = ctx.enter_context(
                tc.tile_pool(name="psum", bufs=5, space="PSUM"))
            psuml = ctx.enter_context(
                tc.tile_pool(name="psuml", bufs=3, space="PSUM"))
            dpool = ctx.enter_context(
                tc.tile_pool(name="dpool", bufs=2, space="DRAM"))

            def sload(ap_dram, shape, dt=bf16, name=None):
                t = singles.tile(shape, dt,
                                 name=name or f"c_{ap_dram.tensor.name}")
                nc.sync.dma_start(out=t[:], in_=ap_dram)
                return t

            # --- weights / constants into SBUF -------------------------------
            w1s_sb = sload(w1s, [CK, H])
            w1r_sb = sload(w1r, [CK, H])
            w2_sb = []
            for k in range(5):
                w2k_sb = singles.tile([H, H], bf16, name=f"w2sb{k}")
                nc.sync.dma_start(out=w2k_sb[:], in_=w2[k])
                w2_sb.append(w2k_sb)
            wp_sb = sload(wp, [H, H], f32)
            wa_sb = sload(wa, [H, 1])
            w11_sb = sload(w11, [H, H], f32)
            w12_sb = sload(w12, [H, H], f32)
            w21_sb = sload(w21, [H, H], f32)
            w22_sb = sload(w22, [H, H], f32)
            w31a_sb = sload(w31a, [H, H], f32)
            w31b_sb = sload(w31b, [H, H], f32)
            w31c_sb = sload(w31c, [H, H], f32)
            w32_sb = sload(w32, [H, H], f32)
            wfco_sb = sload(wfco, [H, O], f32)
            ident_sb = sload(ident, [H, H], f32)
            b1_sb = sload(b1, [H, 1], f32)
            b2p_sb = sload(b2p, [H, 1], f32)
            b11_sb = sload(b11, [H, 1], f32)
            b12_sb = sload(b12, [H, 1], f32)
            b21_sb = sload(b21, [H, 1], f32)
            b22_sb = sload(b22, [H, 1], f32)
            b31_sb = sload(b31, [H, 1], f32)
            b32_sb = sload(b32, [H, 1], f32)
            bfco_sb = sload(bfco, [O, 1], f32)
            rel_ra_sb = sload(rel_r[0:128, :], [128, N], f32, name="rel_ra")
            rel_rb_sb = sload(rel_r[128:EL, :], [EL - 128, N], f32, name="rel_rb")
            rel_sT_sb = sload(rel_sT, [N, EL], f32)
            rel_rT_sb = sload(rel_rT, [N, EL], f32)

            # --- persistent accumulators ------------------------------------
            V_all = singles.tile([H, B * EL], f32, tag="V_all")
            X1T = singles.tile([H, B * EL], f32, tag="X1T")

            def elu(ps, bias_sb, out_sb):
                """out_sb(bf16) = elu(ps + bias)
                = relu(ps+b) + (exp(min(ps+b,0)) - 1)."""
                cols = ps.shape[1]
                tmin = work.tile([ps.shape[0], cols], f32, tag="elu_tmin")
                nc.vector.tensor_scalar(
                    out=tmin[:], in0=ps[:], scalar1=bias_sb[:], scalar2=0.0,
                    op0=Alu.add, op1=Alu.min)
                ex = work.tile([ps.shape[0], cols], f32, tag="elu_ex")
                nc.scalar.activation(ex[:], tmin[:], Act.Exp)
                rl = work.tile([ps.shape[0], cols], f32, tag="elu_rl")
                nc.scalar.activation(rl[:], ps[:], Act.Relu, bias=bias_sb[:])
                nc.vector.scalar_tensor_tensor(
                    out=out_sb, in0=rl[:], scalar=-1.0, in1=ex[:],
                    op0=Alu.add, op1=Alu.add)

            # ================= per-batch edge pipeline =======================
            for b in range(B):
                p1_sb = work.tile([CK, N * PT], bf16, tag="p1_sb")
                nc.sync.dma_start(out=p1_sb[:], in_=p1[b])
                p1r_sb = work.tile([CK, RPC * PT], bf16, tag="p1r_sb")
                nc.sync.dma_start(out=p1r_sb[:], in_=p1r[b])

                Fs = work.tile([H, N * PT], bf16, tag="Fs")
                for s8 in range(N * PT // FSTRIP):
                    fps = psum.tile([H, FSTRIP], f32, tag="ps")
                    nc.tensor.matmul(
                        fps[:], lhsT=w1s_sb[:],
                        rhs=p1_sb[:, s8 * FSTRIP:(s8 + 1) * FSTRIP],
                        start=True, stop=True)
                    nc.scalar.copy(Fs[:, s8 * FSTRIP:(s8 + 1) * FSTRIP], fps[:])
                Fr = work.tile([H, RPC * PT], bf16, tag="Fr")
                frps = psum.tile([H, RPC * PT], f32, tag="ps")
                nc.tensor.matmul(frps[:], lhsT=w1r_sb[:], rhs=p1r_sb[:],
                                 start=True, stop=True)
                nc.scalar.activation(Fr[:], frps[:], Act.Identity, bias=b1_sb[:])

                for rr in range(RPC):
                    col0 = (b * RPC + rr) * N  # edge-column base in V/X1T
                    # G = F_s[all senders] + (F_r[this receiver] + b1):
                    # DMA broadcast-copy of F_r, then DMA accumulate F_s.
                    G = work.tile([H, N * PT], bf16, tag="G")
                    fr_b = Fr[:, rr * PT:(rr + 1) * PT] \
                        .unsqueeze(1).broadcast_to([H, N, PT])
                    nc.sync.dma_start(out=G[:], in_=fr_b)
                    nc.gpsimd.dma_start(out=G[:], in_=Fs[:],
                                        accum_op=Alu.add)
                    # fused maxpool(k=2) + relu: max(G_even, G_odd, 0)
                    Y1 = work.tile([H, N * PL], bf16, tag="Y1")
                    G2 = G[:].rearrange("p (n two) -> p n two", two=2)
                    nc.vector.scalar_tensor_tensor(
                        out=Y1[:].unsqueeze(2), in0=G2[:, :, 0:1],
                        scalar=0.0, in1=G2[:, :, 1:2],
                        op0=Alu.max, op1=Alu.max)
                    # conv2 (bn1 folded) + relu(+b2') -> Y
                    Y = work.tile([H, N * CT], bf16, tag="Y")
                    Y1r = Y1[:].rearrange("p (e t) -> p e t", t=PL)
                    c2ps = [psum.tile([H, C2STRIP], f32, tag="ps",
                                      name=f"c2ps{st}")
                            for st in range(N // C2EDGES)]
                    for k in range(5):
                        for st in range(N // C2EDGES):
                            nc.tensor.matmul(
                                c2ps[st][:],
                                lhsT=w2_sb[k][:],
                                rhs=Y1r[:, st * C2EDGES:(st + 1) * C2EDGES,
                                        k:k + CT],
                                start=(k == 0), stop=(k == 4))
                    for st in range(N // C2EDGES):
                        nc.scalar.activation(
                            Y[:, st * C2STRIP:(st + 1) * C2STRIP],
                            c2ps[st][:], Act.Relu, bias=b2p_sb[:])
                    # attention logits (conva folded; softmax-invariant const
                    # dropped), deinterleaved to [edges, time] layout
                    A_t = work.tile([N, CT], f32, tag="A_t")
                    Lsb = work.tile([1, N * CT], f32, tag="Lsb")
                    for st in range(N // C2EDGES):
                        lps = psuml.tile([1, C2STRIP], f32, tag="lp")
                        nc.tensor.matmul(
                            lps[:], lhsT=wa_sb[:],
                            rhs=Y[:, st * C2STRIP:(st + 1) * C2STRIP],
                            start=True, stop=True)
                        dst = Lsb[:, st * C2STRIP:(st + 1) * C2STRIP]
                        if st % 2 == 0:
                            nc.scalar.copy(dst, lps[:])
                        else:
                            nc.vector.tensor_copy(dst, lps[:])
                    nc.sync.dma_start(out=A_t[:], in_=Lsb[:])
                    # softmax over time (fp32) with 1/44 folded in
                    nmx = work.tile([N, 1], f32, tag="nmx")
                    nc.vector.tensor_reduce(
                        out=nmx[:], in_=A_t[:], axis=mybir.AxisListType.X,
                        op=Alu.max, negate=True)
                    Ex = work.tile([N, CT], f32, tag="Ex")
                    S = work.tile([N, 1], f32, tag="S")
                    nc.scalar.activation(Ex[:], A_t[:], Act.Exp, bias=nmx[:],
                                         accum_out=S[:])
                    S44 = work.tile([N, 1], f32, tag="S44")
                    nc.vector.tensor_scalar_mul(S44[:], S[:], float(CT))
                    rz = work.tile([N, 1], f32, tag="rz")
                    nc.vector.reciprocal(rz[:], S44[:])
                    A_bf = work.tile([N, CT], bf16, tag="A_bf")
                    nc.vector.tensor_scalar_mul(A_bf[:], Ex[:], rz[:])
                    # re-interleave + broadcast across partitions
                    A_dram = dpool.tile([1, N * CT], bf16, tag="A_dram")
                    nc.sync.dma_start(out=A_dram[:], in_=A_bf[:])
                    A_bc = work.tile([H, N * CT], bf16, tag="A_bc")
                    nc.sync.dma_start(
                        out=A_bc[:],
                        in_=A_dram[0:1, :].broadcast_to([H, N * CT]))
                    # weighted temporal mean -> V (pre-convp)
                    Mt = work.tile([H, N * CT], bf16, tag="Mt")
                    nc.vector.tensor_tensor(out=Mt[:], in0=Y[:], in1=A_bc[:],
                                            op=Alu.mult)
                    nc.vector.tensor_reduce(
                        out=V_all[:, col0:col0 + N],
                        in_=Mt[:].rearrange("p (e t) -> p e t", t=CT),
                        axis=mybir.AxisListType.X, op=Alu.add)

            # ================= convp + mlp1 ==================================
            half = B * EL // 2
            for st2 in range(2):
                cs = slice(st2 * half, (st2 + 1) * half)
                zps = psum.tile([H, half], f32, tag="ps")
                nc.tensor.matmul(zps[:], lhsT=wp_sb[:], rhs=V_all[:, cs],
                                 start=True, stop=True)
                xsb = work.tile([H, half], f32, tag="xsb")
                nc.scalar.copy(xsb[:], zps[:])
                h1ps = psum.tile([H, half], f32, tag="ps")
                nc.tensor.matmul(h1ps[:], lhsT=w11_sb[:], rhs=xsb[:],
                                 start=True, stop=True)
                h1sb = work.tile([H, half], f32, tag="h1sb")
                elu(h1ps, b11_sb, h1sb[:])
                h2ps = psum.tile([H, half], f32, tag="ps")
                nc.tensor.matmul(h2ps[:], lhsT=w12_sb[:], rhs=h1sb[:],
                                 start=True, stop=True)
                elu(h2ps, b12_sb, X1T[:, cs])

            # ================= edge2node + AllReduce =========================
            incps = psum.tile([H, B * N], f32, tag="ps")
            chunks = [(0, 128), (128, EL - 128)]
            for b in range(B):
                for j, (c0, cw) in enumerate(chunks):
                    tps = psum.tile([cw, H], f32, tag="ps")
                    nc.tensor.transpose(
                        tps[:], in_=X1T[:, b * EL + c0:b * EL + c0 + cw],
                        identity=ident_sb[:])
                    x1e = work3.tile([cw, H], f32, tag=f"x1e{j}")
                    nc.vector.tensor_copy(x1e[:], tps[:])
                    rel_chunk = rel_ra_sb if j == 0 else rel_rb_sb
                    nc.tensor.matmul(
                        incps[:, b * N:(b + 1) * N], lhsT=x1e[:],
                        rhs=rel_chunk[:], start=(j == 0), stop=(j == 1))
            inc_sb = singles.tile([H, B * N], f32, tag="inc_sb")
            nc.scalar.copy(inc_sb[:], incps[:])
            nc.sync.dma_start(out=cc_in.ap(), in_=inc_sb[:])
            nc.gpsimd.collective_compute(
                "AllReduce", mybir.AluOpType.add,
                replica_groups=[list(range(N_CORES))],
                ins=[cc_in.ap()], outs=[cc_out.ap()])
            incr = singles.tile([H, B * N], f32, tag="incr")
            nc.sync.dma_start(out=incr[:], in_=cc_out.ap())


            # ================= mlp2 (replicated, tiny) =======================
            m2ps = psum.tile([H, B * N], f32, tag="ps")
            nc.tensor.matmul(m2ps[:], lhsT=w21_sb[:], rhs=incr[:],
                             start=True, stop=True)
            m2sb = singles.tile([H, B * N], f32, tag="m2sb")
            elu(m2ps, b21_sb, m2sb[:])
            m2ps2 = psum.tile([H, B * N], f32, tag="ps")
            nc.tensor.matmul(m2ps2[:], lhsT=w22_sb[:], rhs=m2sb[:],
                             start=True, stop=True)
            X2T = singles.tile([H, B * N], f32, tag="X2T")
            elu(m2ps2, b22_sb, X2T[:])

            # ================= node2edge + mlp3 + fco ========================
            for b in range(B):
                x2ps = psum.tile([N, H], f32, tag="ps")
                nc.tensor.transpose(x2ps[:], in_=X2T[:, b * N:(b + 1) * N],
                                    identity=ident_sb[:])
                x2sb = work.tile([N, H], f32, tag="x2sb")
                nc.vector.tensor_copy(x2sb[:], x2ps[:])
                snps = psum.tile([H, EL], f32, tag="ps")
                nc.tensor.matmul(snps[:], lhsT=x2sb[:], rhs=rel_sT_sb[:],
                                 start=True, stop=True)
                snT = work.tile([H, EL], f32, tag="snT")
                nc.scalar.copy(snT[:], snps[:])
                rcps = psum.tile([H, EL], f32, tag="ps")
                nc.tensor.matmul(rcps[:], lhsT=x2sb[:], rhs=rel_rT_sb[:],
                                 start=True, stop=True)
                rcT = work.tile([H, EL], f32, tag="rcT")
                nc.scalar.copy(rcT[:], rcps[:])
                h3ps = psum.tile([H, EL], f32, tag="ps")
                nc.tensor.matmul(h3ps[:], lhsT=w31a_sb[:], rhs=snT[:],
                                 start=True, stop=False)
                nc.tensor.matmul(h3ps[:], lhsT=w31b_sb[:], rhs=rcT[:],
                                 start=False, stop=False)
                nc.tensor.matmul(h3ps[:], lhsT=w31c_sb[:],
                                 rhs=X1T[:, b * EL:(b + 1) * EL],
                                 start=False, stop=True)
                h3sb = work.tile([H, EL], f32, tag="h3sb")
                elu(h3ps, b31_sb, h3sb[:])
                h4ps = psum.tile([H, EL], f32, tag="ps")
                nc.tensor.matmul(h4ps[:], lhsT=w32_sb[:], rhs=h3sb[:],
                                 start=True, stop=True)
                h4sb = work.tile([H, EL], f32, tag="h4sb")
                elu(h4ps, b32_sb, h4sb[:])
                ops = psum.tile([O, EL], f32, tag="ps")
                nc.tensor.matmul(ops[:], lhsT=wfco_sb[:], rhs=h4sb[:],
                                 start=True, stop=True)
                osb = work.tile([O, EL], f32, tag="osb")
                nc.vector.tensor_scalar_add(osb[:], ops[:], bfco_sb[:])
                nc.sync.dma_start(out=y[b].rearrange("e o -> o e"), in_=osb[:])

    nc.compile()
    _PROGRAM_CACHE["nc"] = nc
    return nc


def _host_prep(inputs, rel_rec, rel_send, p, edge_of):
    """Build the per-core input maps + (core, local, global) output mapping."""
    x = inputs.astype(np.float32)
    # im2col of the node time-series: P1[b, c*5+k, n*96+t] = x[b, n, t+k, c]
    win = np.lib.stride_tricks.sliding_window_view(x, 5, axis=2)  # [B,N,96,D,5]
    P1 = win.transpose(0, 3, 4, 1, 2).reshape(B, CK, N, PT)

    a1 = (p["bn1_g"] / np.sqrt(p["bn1_v"] + BN_EPS)).astype(np.float32)
    c1 = (p["bn1_b"] - p["bn1_m"] * a1).astype(np.float32)
    a2 = (p["bn2_g"] / np.sqrt(p["bn2_v"] + BN_EPS)).astype(np.float32)
    c2 = (p["bn2_b"] - p["bn2_m"] * a2).astype(np.float32)

    w1 = p["conv1_w"].astype(np.float32)           # [H, 2D, 5]
    # rows ordered c*5+k to match P1
    W1s = w1[:, :D, :].transpose(1, 2, 0).reshape(CK, H)
    W1r = w1[:, D:, :].transpose(1, 2, 0).reshape(CK, H)

    w2f = p["conv2_w"].astype(np.float32) * a1[None, :, None]   # [o,i,k]
    b2p = p["conv2_b"].astype(np.float32) + np.einsum(
        "oik,i->o", p["conv2_w"].astype(np.float32), c1)
    W2k = [w2f[:, :, k].T.copy() for k in range(5)]             # lhsT [i,o]

    wa = (p["conva_w"][0, :, 0].astype(np.float32) * a2)[:, None]  # [H,1]
    WpT = (p["convp_w"][:, :, 0].astype(np.float32) * a2[None, :]).T  # [i,o]
    bpp = p["convp_b"].astype(np.float32) + \
        p["convp_w"][:, :, 0].astype(np.float32) @ c2

    m1w1 = p["mlp1_w1"].astype(np.float32)
    b11 = p["mlp1_b1"].astype(np.float32) + (bpp / CT) @ m1w1
    b12 = p["mlp1_b2"].astype(np.float32)
    m2w1 = p["mlp2_w1"].astype(np.float32)
    W21 = m2w1 / N
    b21 = p["mlp2_b1"].astype(np.float32)
    b22 = p["mlp2_b2"].astype(np.float32)
    m3w1 = p["mlp3_w1"].astype(np.float32)
    b31 = p["mlp3_b1"].astype(np.float32)
    b32 = p["mlp3_b2"].astype(np.float32)
    bfco = p["fco_b"].astype(np.float32)

    shared = {
        "p1": P1.reshape(B, CK, N * PT).astype(BF16),
        "w1s": W1s.astype(BF16), "w1r": W1r.astype(BF16),
        "wp": WpT.astype(np.float32), "wa": wa.astype(BF16),
        "w11": m1w1.astype(np.float32),
        "w12": p["mlp1_w2"].astype(np.float32),
        "w21": W21.astype(np.float32),
        "w22": p["mlp2_w2"].astype(np.float32),
        "w31a": m3w1[0:H].astype(np.float32),
        "w31b": m3w1[H:2 * H].astype(np.float32),
        "w31c": m3w1[2 * H:3 * H].astype(np.float32),
        "w32": p["mlp3_w2"].astype(np.float32),
        "wfco": p["fco_w"].astype(np.float32),
        "ident": np.eye(H, dtype=np.float32),
        "b1": p["conv1_b"].astype(np.float32)[:, None],
        "b2p": b2p[:, None], "b11": b11[:, None], "b12": b12[:, None],
        "b21": b21[:, None], "b22": b22[:, None],
        "b31": b31[:, None], "b32": b32[:, None],
        "bfco": bfco[:, None],
    }
    for k in range(5):
        shared[f"w2k{k}"] = W2k[k].astype(BF16)

    rr32 = rel_rec.astype(np.float32)
    rs32 = rel_send.astype(np.float32)
    in_maps = []
    out_map = []  # (core, e_loc, e_glob)
    for c in range(N_CORES):
        recvs = list(range(c * RPC, (c + 1) * RPC))
        relr = np.zeros((EL, N), np.float32)
        relsT = np.zeros((N, EL), np.float32)
        relrT = np.zeros((N, EL), np.float32)
        for rr_i, r in enumerate(recvs):
            for s in range(N):
                if s == r:
                    continue
                e_loc = rr_i * N + s
                e_g = edge_of[(r, s)]
                relr[e_loc] = rr32[e_g]
                relsT[:, e_loc] = rs32[e_g]
                relrT[:, e_loc] = rr32[e_g]
                out_map.append((c, e_loc, e_g))
        m = dict(shared)
        m["p1r"] = np.ascontiguousarray(
            P1[:, :, recvs, :]).reshape(B, CK, RPC * PT).astype(BF16)
        m["rel_r"] = relr.astype(np.float32)
        m["rel_sT"] = relsT.astype(np.float32)
        m["rel_rT"] = relrT.astype(np.float32)
        in_maps.append(m)
    return in_maps, out_map


def kernel(**inputs):
    rel_rec = np.asarray(inputs["rel_rec"])
    rel_send = np.asarray(inputs["rel_send"])
    x = np.asarray(inputs["inputs"])
    p = {k: np.asarray(v) for k, v in inputs.items()
         if k not in ("inputs", "rel_rec", "rel_send")}

    edge_of = _nri_structure(rel_rec, rel_send)
    if edge_of is None or x.shape != (B, N, T, D):
        # Inputs without the NRI one-hot structure: fall back to a plain
        # numpy evaluation (correctness path only).
        return _np_forward(x, rel_rec, rel_send, p).astype(np.float32)

    from concourse.bass_utils import run_bass_kernel_spmd

    nc = _build_program()
    in_maps, out_map = _host_prep(x, rel_rec, rel_send, p, edge_of)
    res = run_bass_kernel_spmd(nc, in_maps, list(range(N_CORES)),
                               trace=TRACE)
    if TRACE:
        global LAST_RESULT
        LAST_RESULT = res

    full = np.empty((B, E, O), np.float32)
    for c, e_loc, e_g in out_map:
        full[:, e_g, :] = res.results[c]["y"][:, e_loc, :]
    return full


# revision 14
# speedup vs baseline: 1.2237x; 1.2237x over previous
"""Trainium2 Bass kernel for the NRI CNNEncoder (gnn_message_passing).

Strategy
--------
8-way shard over the edge dimension E=1560: each core owns 5 receiver nodes
x 40 sender slots (the self-edge is computed as padding and discarded on the
host) = 200 local edges x B=4 batches = 800 edge-sequences per core.

Algebraic restructuring (all exact, eval-mode):
- conv1 is linear, so per-edge conv1(concat(send, recv)) = F_s[send] + F_r[recv]
  where F_s/F_r are convolutions of the 160 node sequences with the two halves
  of conv1_w: a 39x compute reduction on conv1.
- BatchNorm (eval) = per-channel positive-scale affine; it commutes with
  maxpool and folds into the following conv/matmul weights (bn1 -> conv2,
  bn2 -> convp/conva).
- convp (1x1) commutes with the attention-weighted temporal pooling, so it is
  applied after pooling: 44x less convp compute.
- ELU is computed as relu(x+b) + exp(min(x+b,0)) - 1; every "-1" is folded
  into the next layer's bias on the host.
- edge2node needs a cross-core reduction: one 80KB fp32 AllReduce.

Matmuls run in bf16 (fp32 PSUM accumulation); softmax in fp32.
"""

import os
import sys
import numpy as np

sys.path.insert(0, "/opt/trn_rl_repo")

import ml_dtypes

BF16 = ml_dtypes.bfloat16

# Problem constants (hardcoded; must match the reference).
B, N, T, D, H, O = 4, 40, 100, 4, 128, 2
E = N * (N - 1)          # 1560
BN_EPS = 1e-5
N_CORES = 8
RPC = N // N_CORES       # receivers per core = 5
EL = RPC * N             # local edges per core (incl. self padding) = 200
PT = T - 4               # conv1 output length = 96
PL = PT // 2             # pooled length = 48
CT = PL - 4              # conv2 output length = 44
CK = 20                  # conv1 contraction = D * K = 4*5
FSTRIP = 480             # F matmul strip (5 nodes x 96)
C2EDGES = 10             # edges per conv2/logits strip
C2STRIP = C2EDGES * CT   # 440


def _np_forward(inputs, rel_rec, rel_send, p):
    """Pure-numpy fp32 replica of the reference (fallback for inputs whose
    rel matrices do not have the NRI one-hot structure)."""
    x32 = inputs.astype(np.float32)
    rr = rel_rec.astype(np.float32)
    rs = rel_send.astype(np.float32)
    xf = x32.reshape(B, N, T * D)
    recv = np.einsum("en,bnf->bef", rr, xf).reshape(B * rr.shape[0], T, D)
    send = np.einsum("en,bnf->bef", rs, xf).reshape(B * rs.shape[0], T, D)
    x = np.concatenate([send.transpose(0, 2, 1), recv.transpose(0, 2, 1)], axis=1)

    def conv1d(x, w, b):
        k = w.shape[2]
        t_out = x.shape[2] - k + 1
        y = np.zeros((x.shape[0], w.shape[0], t_out), np.float32)
        for kk in range(k):
            y += np.einsum("oc,nct->not", w[:, :, kk], x[:, :, kk:kk + t_out])
        return y + b[None, :, None]

    def bn(x, g, b, m, v):
        return (x - m[None, :, None]) / np.sqrt(v[None, :, None] + BN_EPS) \
            * g[None, :, None] + b[None, :, None]

    def elu(x):
        return np.where(x > 0, x, np.expm1(x))

    def mlp(x, w1, b1, w2, b2):
        h = elu(x @ w1 + b1)
        return elu(h @ w2 + b2)

    x = bn(np.maximum(conv1d(x, p["conv1_w"], p["conv1_b"]), 0.0),
           p["bn1_g"], p["bn1_b"], p["bn1_m"], p["bn1_v"])
    n_, c_, t_ = x.shape
    x = x.reshape(n_, c_, t_ // 2, 2).max(axis=-1)
    x = bn(np.maximum(conv1d(x, p["conv2_w"], p["conv2_b"]), 0.0),
           p["bn2_g"], p["bn2_b"], p["bn2_m"], p["bn2_v"])
    pred = conv1d(x, p["convp_w"], p["convp_b"])
    a = conv1d(x, p["conva_w"], p["conva_b"])
    a = np.exp(a - a.max(axis=2, keepdims=True))
    a = a / a.sum(axis=2, keepdims=True)
    x = (pred * a).mean(axis=2).reshape(B, -1, H)
    x = mlp(x, p["mlp1_w1"], p["mlp1_b1"], p["mlp1_w2"], p["mlp1_b2"])
    x_skip = x
    inc = np.einsum("en,beh->bnh", rr, x) / N
    x = mlp(inc, p["mlp2_w1"], p["mlp2_b1"], p["mlp2_w2"], p["mlp2_b2"])
    sn = np.einsum("en,bnh->beh", rs, x)
    rc = np.einsum("en,bnh->beh", rr, x)
    x = np.concatenate([sn, rc, x_skip], axis=2)
    x = mlp(x, p["mlp3_w1"], p["mlp3_b1"], p["mlp3_w2"], p["mlp3_b2"])
    return x @ p["fco_w"] + p["fco_b"]


def _nri_structure(rel_rec, rel_send):
    """If (rel_rec, rel_send) are the NRI fully-connected one-hot matrices,
    return edge_of[r][s] -> global edge index; else None."""
    if rel_rec.shape != (E, N) or rel_send.shape != (E, N):
        return None
    rec_i = np.argmax(rel_rec, axis=1)
    snd_i = np.argmax(rel_send, axis=1)
    eye = np.eye(N, dtype=rel_rec.dtype)
    if not (np.array_equal(rel_rec, eye[rec_i]) and
            np.array_equal(rel_send, eye[snd_i])):
        return None
    edge_of = {}
    for e in range(E):
        r, s = int(rec_i[e]), int(snd_i[e])
        if r == s or (r, s) in edge_of:
            return None
        edge_of[(r, s)] = e
    if len(edge_of) != E:
        return None
    return edge_of


_PROGRAM_CACHE = {}
TRACE = False          # test harness sets True to collect NTFF exec time
LAST_RESULT = None     # BassKernelResults of the last run (when TRACE)


def _build_program():
    """Build + compile the SPMD Bass program (cached per process)."""
    if "nc" in _PROGRAM_CACHE:
        return _PROGRAM_CACHE["nc"]

    import concourse.bacc as bacc
    import concourse.tile as tile
    from concourse import mybir
    from contextlib import ExitStack

    f32 = mybir.dt.float32
    bf16 = mybir.dt.bfloat16
    Alu = mybir.AluOpType
    Act = mybir.ActivationFunctionType

    nc = bacc.Bacc("TRN2", target_bir_lowering=False, debug=False,
                   num_devices=N_CORES)

    def din(name, shape, dt=bf16):
        return nc.dram_tensor(name, shape, dt, kind="ExternalInput").ap()

    p1 = din("p1", [B, CK, N * PT])
    p1r = din("p1r", [B, CK, RPC * PT])
    rel_r = din("rel_r", [EL, N], f32)
    rel_sT = din("rel_sT", [N, EL], f32)
    rel_rT = din("rel_rT", [N, EL], f32)
    w1s = din("w1s", [CK, H])
    w1r = din("w1r", [CK, H])
    w2 = [din(f"w2k{k}", [H, H]) for k in range(5)]
    wp = din("wp", [H, H], f32)
    wa = din("wa", [H, 1])
    w11 = din("w11", [H, H], f32)
    w12 = din("w12", [H, H], f32)
    w21 = din("w21", [H, H], f32)
    w22 = din("w22", [H, H], f32)
    w31a = din("w31a", [H, H], f32)
    w31b = din("w31b", [H, H], f32)
    w31c = din("w31c", [H, H], f32)
    w32 = din("w32", [H, H], f32)
    wfco = din("wfco", [H, O], f32)
    ident = din("ident", [H, H], f32)
    b1 = din("b1", [H, 1], f32)
    b2p = din("b2p", [H, 1], f32)
    b11 = din("b11", [H, 1], f32)
    b12 = din("b12", [H, 1], f32)
    b21 = din("b21", [H, 1], f32)
    b22 = din("b22", [H, 1], f32)
    b31 = din("b31", [H, 1], f32)
    b32 = din("b32", [H, 1], f32)
    bfco = din("bfco", [O, 1], f32)

    y = nc.dram_tensor("y", [B, EL, O], f32, kind="ExternalOutput").ap()
    cc_in = nc.dram_tensor("cc_in", [H, B * N], f32)
    cc_out = nc.dram_tensor("cc_out", [H, B * N], f32, addr_space="Shared")

    with tile.TileContext(nc) as tc:
        with ExitStack() as ctx:
            singles = ctx.enter_context(tc.tile_pool(name="singles", bufs=1))
            work = ctx.enter_context(tc.tile_pool(name="work", bufs=2))
            work3 = ctx.enter_context(tc.tile_pool(name="work3", bufs=3))
            psum = ctx.enter_context(
                tc.tile_pool(name="psum", bufs=5, space="PSUM"))
            psuml = ctx.enter_context(
                tc.tile_pool(name="psuml", bufs=3, space="PSUM"))
            dpool = ctx.enter_context(
                tc.tile_pool(name="dpool", bufs=2, space="DRAM"))

            def sload(ap_dram, shape, dt=bf16, name=None):
                t = singles.tile(shape, dt,
                                 name=name or f"c_{ap_dram.tensor.name}")
                nc.sync.dma_start(out=t[:], in_=ap_dram)
                return t

            # --- weights / constants into SBUF -------------------------------
            w1s_sb = sload(w1s, [CK, H])
            w1r_sb = sload(w1r, [CK, H])
            w2_sb = []
            for k in range(5):
                w2k_sb = singles.tile([H, H], bf16, name=f"w2sb{k}")
                nc.sync.dma_start(out=w2k_sb[:], in_=w2[k])
                w2_sb.append(w2k_sb)
            wp_sb = sload(wp, [H, H], f32)
            wa_sb = sload(wa, [H, 1])
            w11_sb = sload(w11, [H, H], f32)
            w12_sb = sload(w12, [H, H], f32)
            w21_sb = sload(w21, [H, H], f32)
            w22_sb = sload(w22, [H, H], f32)
            w31a_sb = sload(w31a, [H, H], f32)
            w31b_sb = sload(w31b, [H, H], f32)
            w31c_sb = sload(w31c, [H, H], f32)
            w32_sb = sload(w32, [H, H], f32)
            wfco_sb = sload(wfco, [H, O], f32)
            ident_sb = sload(ident, [H, H], f32)
            b1_sb = sload(b1, [H, 1], f32)
            b2p_sb = sload(b2p, [H, 1], f32)
            b11_sb = sload(b11, [H, 1], f32)
            b12_sb = sload(b12, [H, 1], f32)
            b21_sb = sload(b21, [H, 1], f32)
            b22_sb = sload(b22, [H, 1], f32)
            b31_sb = sload(b31, [H, 1], f32)
            b32_sb = sload(b32, [H, 1], f32)
            bfco_sb = sload(bfco, [O, 1], f32)
            rel_ra_sb = sload(rel_r[0:128, :], [128, N], f32, name="rel_ra")
            rel_rb_sb = sload(rel_r[128:EL, :], [EL - 128, N], f32, name="rel_rb")
            rel_sT_sb = sload(rel_sT, [N, EL], f32)
            rel_rT_sb = sload(rel_rT, [N, EL], f32)

            # --- persistent accumulators ------------------------------------
            V_all = singles.tile([H, B * EL], f32, tag="V_all")
            X1T = singles.tile([H, B * EL], f32, tag="X1T")

            def elu(ps, bias_sb, out_sb):
                """out_sb(bf16) = elu(ps + bias)
                = relu(ps+b) + (exp(min(ps+b,0)) - 1)."""
                cols = ps.shape[1]
                tmin = work.tile([ps.shape[0], cols], f32, tag="elu_tmin")
                nc.vector.tensor_scalar(
                    out=tmin[:], in0=ps[:], scalar1=bias_sb[:], scalar2=0.0,
                    op0=Alu.add, op1=Alu.min)
                ex = work.tile([ps.shape[0], cols], f32, tag="elu_ex")
                nc.scalar.activation(ex[:], tmin[:], Act.Exp)
                rl = work.tile([ps.shape[0], cols], f32, tag="elu_rl")
                nc.scalar.activation(rl[:], ps[:], Act.Relu, bias=bias_sb[:])
                nc.vector.scalar_tensor_tensor(
                    out=out_sb, in0=rl[:], scalar=-1.0, in1=ex[:],
                    op0=Alu.add, op1=Alu.add)

            # ================= per-batch edge pipeline =======================
            for b in range(B):
                p1_sb = work.tile([CK, N * PT], bf16, tag="p1_sb")
                nc.sync.dma_start(out=p1_sb[:], in_=p1[b])
                p1r_sb = work.tile([CK, RPC * PT], bf16, tag="p1r_sb")
                nc.sync.dma_start(out=p1r_sb[:], in_=p1r[b])

                Fs = work.tile([H, N * PT], bf16, tag="Fs")
                for s8 in range(N * PT // FSTRIP):
                    fps = psum.tile([H, FSTRIP], f32, tag="ps")
                    nc.tensor.matmul(
                        fps[:], lhsT=w1s_sb[:],
                        rhs=p1_sb[:, s8 * FSTRIP:(s8 + 1) * FSTRIP],
                        start=True, stop=True)
                    nc.scalar.copy(Fs[:, s8 * FSTRIP:(s8 + 1) * FSTRIP], fps[:])
                Fr = work.tile([H, RPC * PT], bf16, tag="Fr")
                frps = psum.tile([H, RPC * PT], f32, tag="ps")
                nc.tensor.matmul(frps[:], lhsT=w1r_sb[:], rhs=p1r_sb[:],
                                 start=True, stop=True)
                nc.scalar.activation(Fr[:], frps[:], Act.Identity, bias=b1_sb[:])

                for rr in range(RPC):
                    col0 = (b * RPC + rr) * N  # edge-column base in V/X1T
                    # G = F_s[all senders] + (F_r[this receiver] + b1):
                    # DMA broadcast-copy of F_r, then DMA accumulate F_s.
                    G = work.tile([H, N * PT], bf16, tag="G")
                    fr_b = Fr[:, rr * PT:(rr + 1) * PT] \
                        .unsqueeze(1).broadcast_to([H, N, PT])
                    nc.vector.tensor_tensor(
                        out=G[:].rearrange("p (n t) -> p n t", t=PT),
                        in0=Fs[:].rearrange("p (n t) -> p n t", t=PT),
                        in1=fr_b, op=Alu.add)
                    # fused maxpool(k=2) + relu: max(G_even, G_odd, 0)
                    Y1 = work.tile([H, N * PL], bf16, tag="Y1")
                    G2 = G[:].rearrange("p (n two) -> p n two", two=2)
                    nc.vector.scalar_tensor_tensor(
                        out=Y1[:].unsqueeze(2), in0=G2[:, :, 0:1],
                        scalar=0.0, in1=G2[:, :, 1:2],
                        op0=Alu.max, op1=Alu.max)
                    # conv2 (bn1 folded) + relu(+b2') -> Y
                    Y = work.tile([H, N * CT], bf16, tag="Y")
                    Y1r = Y1[:].rearrange("p (e t) -> p e t", t=PL)
                    c2ps = [psum.tile([H, C2STRIP], f32, tag="ps",
                                      name=f"c2ps{st}")
                            for st in range(N // C2EDGES)]
                    for k in range(5):
                        for st in range(N // C2EDGES):
                            nc.tensor.matmul(
                                c2ps[st][:],
                                lhsT=w2_sb[k][:],
                                rhs=Y1r[:, st * C2EDGES:(st + 1) * C2EDGES,
                                        k:k + CT],
                                start=(k == 0), stop=(k == 4))
                    for st in range(N // C2EDGES):
                        nc.scalar.activation(
                            Y[:, st * C2STRIP:(st + 1) * C2STRIP],
                            c2ps[st][:], Act.Relu, bias=b2p_sb[:])
                    # attention logits (conva folded; softmax-invariant const
                    # dropped), deinterleaved to [edges, time] layout
                    A_t = work.tile([N, CT], f32, tag="A_t")
                    Lsb = work.tile([1, N * CT], f32, tag="Lsb")
                    for st in range(N // C2EDGES):
                        lps = psuml.tile([1, C2STRIP], f32, tag="lp")
                        nc.tensor.matmul(
                            lps[:], lhsT=wa_sb[:],
                            rhs=Y[:, st * C2STRIP:(st + 1) * C2STRIP],
                            start=True, stop=True)
                        dst = Lsb[:, st * C2STRIP:(st + 1) * C2STRIP]
                        if st % 2 == 0:
                            nc.scalar.copy(dst, lps[:])
                        else:
                            nc.vector.tensor_copy(dst, lps[:])
                    nc.sync.dma_start(out=A_t[:], in_=Lsb[:])
                    # softmax over time (fp32) with 1/44 folded in
                    nmx = work.tile([N, 1], f32, tag="nmx")
                    nc.vector.tensor_reduce(
                        out=nmx[:], in_=A_t[:], axis=mybir.AxisListType.X,
                        op=Alu.max, negate=True)
                    Ex = work.tile([N, CT], f32, tag="Ex")
                    S = work.tile([N, 1], f32, tag="S")
                    nc.scalar.activation(Ex[:], A_t[:], Act.Exp, bias=nmx[:],
                                         accum_out=S[:])
                    S44 = work.tile([N, 1], f32, tag="S44")
                    nc.vector.tensor_scalar_mul(S44[:], S[:], float(CT))
                    rz = work.tile([N, 1], f32, tag="rz")
                    nc.vector.reciprocal(rz[:], S44[:])
                    A_bf = work.tile([N, CT], bf16, tag="A_bf")
                    nc.vector.tensor_scalar_mul(A_bf[:], Ex[:], rz[:])
                    # re-interleave + broadcast across partitions
                    A_dram = dpool.tile([1, N * CT], bf16, tag="A_dram")
                    nc.gpsimd.dma_start(out=A_dram[:], in_=A_bf[:])
                    A_bc = work.tile([H, N * CT], bf16, tag="A_bc")
                    nc.scalar.dma_start(
                        out=A_bc[:],
                        in_=A_dram[0:1, :].broadcast_to([H, N * CT]))
                    # weighted temporal mean -> V (pre-convp)
                    Mt = work.tile([H, N * CT], bf16, tag="Mt")
                    nc.vector.tensor_tensor(out=Mt[:], in0=Y[:], in1=A_bc[:],
                                            op=Alu.mult)
                    nc.vector.tensor_reduce(
                        out=V_all[:, col0:col0 + N],
                        in_=Mt[:].rearrange("p (e t) -> p e t", t=CT),
                        axis=mybir.AxisListType.X, op=Alu.add)

            # ================= convp + mlp1 ==================================
            half = B * EL // 2
            for st2 in range(2):
                cs = slice(st2 * half, (st2 + 1) * half)
                zps = psum.tile([H, half], f32, tag="ps")
                nc.tensor.matmul(zps[:], lhsT=wp_sb[:], rhs=V_all[:, cs],
                                 start=True, stop=True)
                xsb = work.tile([H, half], f32, tag="xsb")
                nc.scalar.copy(xsb[:], zps[:])
                h1ps = psum.tile([H, half], f32, tag="ps")
                nc.tensor.matmul(h1ps[:], lhsT=w11_sb[:], rhs=xsb[:],
                                 start=True, stop=True)
                h1sb = work.tile([H, half], f32, tag="h1sb")
                elu(h1ps, b11_sb, h1sb[:])
                h2ps = psum.tile([H, half], f32, tag="ps")
                nc.tensor.matmul(h2ps[:], lhsT=w12_sb[:], rhs=h1sb[:],
                                 start=True, stop=True)
                elu(h2ps, b12_sb, X1T[:, cs])

            # ================= edge2node + AllReduce =========================
            incps = psum.tile([H, B * N], f32, tag="ps")
            chunks = [(0, 128), (128, EL - 128)]
            for b in range(B):
                for j, (c0, cw) in enumerate(chunks):
                    tps = psum.tile([cw, H], f32, tag="ps")
                    nc.tensor.transpose(
                        tps[:], in_=X1T[:, b * EL + c0:b * EL + c0 + cw],
                        identity=ident_sb[:])
                    x1e = work3.tile([cw, H], f32, tag=f"x1e{j}")
                    nc.vector.tensor_copy(x1e[:], tps[:])
                    rel_chunk = rel_ra_sb if j == 0 else rel_rb_sb
                    nc.tensor.matmul(
                        incps[:, b * N:(b + 1) * N], lhsT=x1e[:],
                        rhs=rel_chunk[:], start=(j == 0), stop=(j == 1))
            inc_sb = singles.tile([H, B * N], f32, tag="inc_sb")
            nc.scalar.copy(inc_sb[:], incps[:])
            nc.gpsimd.dma_start(out=cc_in.ap(), in_=inc_sb[:])
            nc.gpsimd.collective_compute(
                "AllReduce", mybir.AluOpType.add,
                replica_groups=[list(range(N_CORES))],
                ins=[cc_in.ap()], outs=[cc_out.ap()])
            incr = singles.tile([H, B * N], f32, tag="incr")
            nc.gpsimd.dma_start(out=incr[:], in_=cc_out.ap())


            # ================= mlp2 (replicated, tiny) =======================
            m2ps = psum.tile([H, B * N], f32, tag="ps")
            nc.tensor.matmul(m2ps[:], lhsT=w21_sb[:], rhs=incr[:],
                             start=True, stop=True)
            m2sb = singles.tile([H, B * N], f32, tag="m2sb")
            elu(m2ps, b21_sb, m2sb[:])
            m2ps2 = psum.tile([H, B * N], f32, tag="ps")
            nc.tensor.matmul(m2ps2[:], lhsT=w22_sb[:], rhs=m2sb[:],
                             start=True, stop=True)
            X2T = singles.tile([H, B * N], f32, tag="X2T")
            elu(m2ps2, b22_sb, X2T[:])

            # ================= node2edge + mlp3 + fco ========================
            for b in range(B):
                x2ps = psum.tile([N, H], f32, tag="ps")
                nc.tensor.transpose(x2ps[:], in_=X2T[:, b * N:(b + 1) * N],
                                    identity=ident_sb[:])
                x2sb = work.tile([N, H], f32, tag="x2sb")
                nc.vector.tensor_copy(x2sb[:], x2ps[:])
                snps = psum.tile([H, EL], f32, tag="ps")
                nc.tensor.matmul(snps[:], lhsT=x2sb[:], rhs=rel_sT_sb[:],
                                 start=True, stop=True)
                snT = work.tile([H, EL], f32, tag="snT")
                nc.scalar.copy(snT[:], snps[:])
                rcps = psum.tile([H, EL], f32, tag="ps")
                nc.tensor.matmul(rcps[:], lhsT=x2sb[:], rhs=rel_rT_sb[:],
                                 start=True, stop=True)
                rcT = work.tile([H, EL], f32, tag="rcT")
                nc.scalar.copy(rcT[:], rcps[:])
                h3ps = psum.tile([H, EL], f32, tag="ps")
                nc.tensor.matmul(h3ps[:], lhsT=w31a_sb[:], rhs=snT[:],
                                 start=True, stop=False)
                nc.tensor.matmul(h3ps[:], lhsT=w31b_sb[:], rhs=rcT[:],
                                 start=False, stop=False)
                nc.tensor.matmul(h3ps[:], lhsT=w31c_sb[:],
                                 rhs=X1T[:, b * EL:(b + 1) * EL],
                                 start=False, stop=True)
                h3sb = work.tile([H, EL], f32, tag="h3sb")
                elu(h3ps, b31_sb, h3sb[:])
                h4ps = psum.tile([H, EL], f32, tag="ps")
                nc.tensor.matmul(h4ps[:], lhsT=w32_sb[:], rhs=h3sb[:],
                                 start=True, stop=True)
                h4sb = work.tile([H, EL], f32, tag="h4sb")
                elu(h4ps, b32_sb, h4sb[:])
                ops = psum.tile([O, EL], f32, tag="ps")
                nc.tensor.matmul(ops[:], lhsT=wfco_sb[:], rhs=h4sb[:],
                                 start=True, stop=True)
                osb = work.tile([O, EL], f32, tag="osb")
                nc.vector.tensor_scalar_add(osb[:], ops[:], bfco_sb[:])
                nc.gpsimd.dma_start(out=y[b].rearrange("e o -> o e"), in_=osb[:])

    nc.compile()
    _PROGRAM_CACHE["nc"] = nc
    return nc


def _host_prep(inputs, rel_rec, rel_send, p, edge_of):
    """Build the per-core input maps + (core, local, global) output mapping."""
    x = inputs.astype(np.float32)
    # im2col of the node time-series: P1[b, c*5+k, n*96+t] = x[b, n, t+k, c]
    win = np.lib.stride_tricks.sliding_window_view(x, 5, axis=2)  # [B,N,96,D,5]
    P1 = win.transpose(0, 3, 4, 1, 2).reshape(B, CK, N, PT)

    a1 = (p["bn1_g"] / np.sqrt(p["bn1_v"] + BN_EPS)).astype(np.float32)
    c1 = (p["bn1_b"] - p["bn1_m"] * a1).astype(np.float32)
    a2 = (p["bn2_g"] / np.sqrt(p["bn2_v"] + BN_EPS)).astype(np.float32)
    c2 = (p["bn2_b"] - p["bn2_m"] * a2).astype(np.float32)

    w1 = p["conv1_w"].astype(np.float32)           # [H, 2D, 5]
    # rows ordered c*5+k to match P1
    W1s = w1[:, :D, :].transpose(1, 2, 0).reshape(CK, H)
    W1r = w1[:, D:, :].transpose(1, 2, 0).reshape(CK, H)

    w2f = p["conv2_w"].astype(np.float32) * a1[None, :, None]   # [o,i,k]
    b2p = p["conv2_b"].astype(np.float32) + np.einsum(
        "oik,i->o", p["conv2_w"].astype(np.float32), c1)
    W2k = [w2f[:, :, k].T.copy() for k in range(5)]             # lhsT [i,o]

    wa = (p["conva_w"][0, :, 0].astype(np.float32) * a2)[:, None]  # [H,1]
    WpT = (p["convp_w"][:, :, 0].astype(np.float32) * a2[None, :]).T  # [i,o]
    bpp = p["convp_b"].astype(np.float32) + \
        p["convp_w"][:, :, 0].astype(np.float32) @ c2

    m1w1 = p["mlp1_w1"].astype(np.float32)
    b11 = p["mlp1_b1"].astype(np.float32) + (bpp / CT) @ m1w1
    b12 = p["mlp1_b2"].astype(np.float32)
    m2w1 = p["mlp2_w1"].astype(np.float32)
    W21 = m2w1 / N
    b21 = p["mlp2_b1"].astype(np.float32)
    b22 = p["mlp2_b2"].astype(np.float32)
    m3w1 = p["mlp3_w1"].astype(np.float32)
    b31 = p["mlp3_b1"].astype(np.float32)
    b32 = p["mlp3_b2"].astype(np.float32)
    bfco = p["fco_b"].astype(np.float32)

    shared = {
        "p1": P1.reshape(B, CK, N * PT).astype(BF16),
        "w1s": W1s.astype(BF16), "w1r": W1r.astype(BF16),
        "wp": WpT.astype(np.float32), "wa": wa.astype(BF16),
        "w11": m1w1.astype(np.float32),
        "w12": p["mlp1_w2"].astype(np.float32),
        "w21": W21.astype(np.float32),
        "w22": p["mlp2_w2"].astype(np.float32),
        "w31a": m3w1[0:H].astype(np.float32),
        "w31b": m3w1[H:2 * H].astype(np.float32),
        "w31c": m3w1[2 * H:3 * H].astype(np.float32),
        "w32": p["mlp3_w2"].astype(np.float32),
        "wfco": p["fco_w"].astype(np.float32),
        "ident": np.eye(H, dtype=np.float32),
        "b1": p["conv1_b"].astype(np.float32)[:, None],
        "b2p": b2p[:, None], "b11": b11[:, None], "b12": b12[:, None],
        "b21": b21[:, None], "b22": b22[:, None],
        "b31": b31[:, None], "b32": b32[:, None],
        "bfco": bfco[:, None],
    }
    for k in range(5):
        shared[f"w2k{k}"] = W2k[k].astype(BF16)

    rr32 = rel_rec.astype(np.float32)
    rs32 = rel_send.astype(np.float32)
    in_maps = []
    out_map = []  # (core, e_loc, e_glob)
    for c in range(N_CORES):
        recvs = list(range(c * RPC, (c + 1) * RPC))
        relr = np.zeros((EL, N), np.float32)
        relsT = np.zeros((N, EL), np.float32)
        relrT = np.zeros((N, EL), np.float32)
        for rr_i, r in enumerate(recvs):
            for s in range(N):
                if s == r:
                    continue
                e_loc = rr_i * N + s
                e_g = edge_of[(r, s)]
                relr[e_loc] = rr32[e_g]
                relsT[:, e_loc] = rs32[e_g]
                relrT[:, e_loc] = rr32[e_g]
                out_map.append((c, e_loc, e_g))
        m = dict(shared)
        m["p1r"] = np.ascontiguousarray(
            P1[:, :, recvs, :]).reshape(B, CK, RPC * PT).astype(BF16)
        m["rel_r"] = relr.astype(np.float32)
        m["rel_sT"] = relsT.astype(np.float32)
        m["rel_rT"] = relrT.astype(np.float32)
        in_maps.append(m)
    return in_maps, out_map


def kernel(**inputs):
    rel_rec = np.asarray(inputs["rel_rec"])
    rel_send = np.asarray(inputs["rel_send"])
    x = np.asarray(inputs["inputs"])
    p = {k: np.asarray(v) for k, v in inputs.items()
         if k not in ("inputs", "rel_rec", "rel_send")}

    edge_of = _nri_structure(rel_rec, rel_send)
    if edge_of is None or x.shape != (B, N, T, D):
        # Inputs without the NRI one-hot structure: fall back to a plain
        # numpy evaluation (correctness path only).
        return _np_forward(x, rel_rec, rel_send, p).astype(np.float32)

    from concourse.bass_utils import run_bass_kernel_spmd

    nc = _build_program()
    in_maps, out_map = _host_prep(x, rel_rec, rel_send, p, edge_of)
    res = run_bass_kernel_spmd(nc, in_maps, list(range(N_CORES)),
                               trace=TRACE)
    if TRACE:
        global LAST_RESULT
        LAST_RESULT = res

    full = np.empty((B, E, O), np.float32)
    for c, e_loc, e_g in out_map:
        full[:, e_g, :] = res.results[c]["y"][:, e_loc, :]
    return full
